# revision 31
# baseline (speedup 1.0000x reference)
"""Trainium2 Bass kernel: GPT-2-style causal multi-head attention (bf16 v3).

Problem: B=4, S=2048, D=1024, H=16 heads (head_dim 64), fp32 reference.
  q/k/v = x @ W{q,k,v} + b{q,k,v}; causal softmax attention; out = attn @ Wo + bo.

Sharding (8 cores): tensor-parallel over heads - each core owns 2 heads
(J=128 feature dims). Wq/Wk/Wv column-sliced, Wo row-sliced per core. Each
core computes a partial o_proj output (transposed, [D, B*S], bf16); the host
sums the 8 partials in fp32, transposes, and adds bo.

v3 (from v2, 562.9 us): the attention inner loop was dependency-stalled
(scores -> exp[ACT] -> mask[gpsimd] -> PV chain made the in-order PE wait
~500 ns/iter and kept it at mid p-state). Fixes:
  * software pipelining: scores emitted LA=3 iterations ahead of PV.
  * causal mask off the critical path: PV of a diagonal tile is split into
    an unmasked part (issues right after exp) + the masked 128 cols.
  * fine-grained interleave: QKV matmuls of batch b+1 and o_proj matmuls of
    batch b-1 are emitted as filler between attention iterations of batch b,
    so the PE never idles (and stays at full p-state).
  * ve copies batched into single strided 4D-AP copies (was 128 tiny CASTs).
  * plain tensor_copy for q/k/v PSUM->SBUF when biases are all zero
    (tensor_scalar_add costs ~3x a copy on DVE); bias variant kept.
"""

import sys

sys.path.insert(0, "/opt/trn_rl_repo")

import numpy as np

import concourse.bass as bass
import concourse.bacc as bacc
import concourse.tile as tile
import concourse.mybir as mybir
from concourse.bass_utils import run_bass_kernel_spmd

F32 = mybir.dt.float32
BF16 = mybir.dt.bfloat16

B, S, D, H = 4, 2048, 1024, 16
HD = D // H  # 64
N_CORES = 8
HPC = H // N_CORES  # heads per core = 2
J = HPC * HD  # per-core feature dims = 128
BS = B * S  # 8192
NB = S // 128  # 16 s-blocks per batch
NC = S // 512  # 4 chunks of 512 per batch
LA = 3  # scores lookahead (must be <= ps_b bufs)


class FillQueue:
    """Queue of single-instruction emitters with chain boundaries.

    Items: ("u", fn) plain unit, ("b", fn) chain begin, ("e", fn) chain end.
    pop(n) emits n units; drain_chain() finishes an open chain so PSUM ring
    slots held by a partially-emitted accumulation chain get released before
    an out-of-band allocation (avoids tile-scheduler deadlock).
    """

    def __init__(self):
        self.items = []
        self.pos = 0
        self.in_chain = False

    def push(self, kind, fn):
        self.items.append((kind, fn))

    def _step(self):
        kind, fn = self.items[self.pos]
        self.pos += 1
        fn()
        if kind == "b":
            self.in_chain = True
        elif kind == "e":
            self.in_chain = False

    def pop(self, n):
        for _ in range(n):
            if self.pos >= len(self.items):
                return
            self._step()

    def drain_chain(self):
        while self.in_chain and self.pos < len(self.items):
            self._step()

    def drain_all(self):
        while self.pos < len(self.items):
            self._step()


def build_kernel(with_bias=False, n_batches=B, debug_outs=()):
    nc = bacc.Bacc(
        "TRN2", target_bir_lowering=False, debug=False, enable_asserts=False,
        num_devices=N_CORES,
    )

    t = {}
    t["xt"] = nc.dram_tensor("xt", [D, BS], BF16, kind="ExternalInput").ap()
    for w in ("wq", "wk", "wv"):
        t[w] = nc.dram_tensor(w, [128, D], BF16, kind="ExternalInput").ap()
    t["wo"] = nc.dram_tensor("wo", [J, D], BF16, kind="ExternalInput").ap()
    for bn in ("bq", "bk", "bv"):
        t[bn] = nc.dram_tensor(bn, [J], F32, kind="ExternalInput").ap()
    t["cident"] = nc.dram_tensor("cident", [128, 128], BF16,
                                 kind="ExternalInput").ap()
    t["cmask"] = nc.dram_tensor("cmask", [128, 128], BF16,
                                kind="ExternalInput").ap()
    t["cones"] = nc.dram_tensor("cones", [128, 32], BF16,
                                kind="ExternalInput").ap()
    t["csel"] = nc.dram_tensor("csel", [8, 512], BF16,
                               kind="ExternalInput").ap()
    t["out"] = nc.dram_tensor("out_t", [D, BS], BF16, kind="ExternalOutput").ap()
    for dbg in debug_outs:
        t[f"dbg_{dbg}"] = nc.dram_tensor(
            f"dbg_{dbg}", [128, 4096], BF16, kind="ExternalOutput").ap()

    with tile.TileContext(nc) as tc:
        _emit(tc, nc, t, with_bias, n_batches, debug_outs)

    nc.compile()
    return nc


def _emit(tc, nc, td, with_bias, n_batches=B, debug_outs=()):
    from contextlib import ExitStack

    ctx = ExitStack()
    with ctx:
        const = ctx.enter_context(tc.tile_pool(name="const", bufs=1))
        wpool = ctx.enter_context(tc.tile_pool(name="w", bufs=1))
        xpool = ctx.enter_context(tc.tile_pool(name="x", bufs=12))
        qkvpool = ctx.enter_context(tc.tile_pool(name="qkv", bufs=2))
        vepool = ctx.enter_context(tc.tile_pool(name="ve", bufs=2))
        ptpool = ctx.enter_context(tc.tile_pool(name="pt", bufs=8))
        aoupool = ctx.enter_context(tc.tile_pool(name="aou", bufs=20))
        aotpool = ctx.enter_context(tc.tile_pool(name="aot", bufs=2))
        nrmpool = ctx.enter_context(tc.tile_pool(name="nrm", bufs=2))
        stgpool = ctx.enter_context(tc.tile_pool(name="stg", bufs=3))
        # PSUM banks: ps_a 2 + ps_b 3 + ps_c 3 = 8
        ps_a = ctx.enter_context(tc.tile_pool(name="ps_a", bufs=2, space="PSUM"))
        ps_b = ctx.enter_context(tc.tile_pool(name="ps_b", bufs=3, space="PSUM"))
        ps_c = ctx.enter_context(tc.tile_pool(name="ps_c", bufs=3, space="PSUM"))

        # --- constants (host-precomputed) --------------------------------
        ident_b = const.tile([128, 128], BF16, tag="ident_b")
        nc.sync.dma_start(ident_b[:], td["cident"][:, :])
        mask_b = const.tile([128, 128], BF16, tag="mask_b")
        nc.sync.dma_start(mask_b[:], td["cmask"][:, :])
        ones32 = const.tile([128, 32], BF16, tag="ones32")
        nc.sync.dma_start(ones32[:], td["cones"][:, :])
        sel = const.tile([8, 512], BF16, tag="sel")
        nc.sync.dma_start(sel[:], td["csel"][:, :])

        # --- weights -----------------------------------------------------
        w_tiles = {}
        for name, key in (("q", "wq"), ("k", "wk"), ("v", "wv")):
            wt = wpool.tile([128, D], BF16, tag=f"w{name}")
            nc.sync.dma_start(wt[:], td[key][:, :])
            w_tiles[name] = wt
        wo_t = wpool.tile([J, D], BF16, tag="wo")
        nc.sync.dma_start(wo_t[:], td["wo"][:, :])

        bias = {}
        for name, key in (("q", "bq"), ("k", "bk"), ("v", "bv")):
            bt = const.tile([J, 1], F32, tag=f"b{name}")
            nc.sync.dma_start(bt[:], td[key].rearrange("(p o) -> p o", o=1))
            bias[name] = bt

        # --- per-batch building blocks -----------------------------------

        def load_xt(b):
            xt = []
            for ib in range(8):
                xti = xpool.tile([128, S], BF16, tag="xt", name="xt")
                nc.sync.dma_start(
                    xti[:], td["xt"][ib * 128:(ib + 1) * 128,
                                     b * S: b * S + S])
                xt.append(xti)
            return xt

        def qkv_jobs(fq, b, xt, proj_out):
            """Push the 12 projection chain jobs for batch b into fq.
            Tiles are allocated lazily inside the closures (pool.tile() is a
            program-order event; eager allocation would deadlock the rings).
            """
            hold = {}

            def mk_proj_alloc(nm):
                def f():
                    proj_out[nm] = qkvpool.tile(
                        [J, S], BF16, tag=f"{nm}t", name=f"{nm}t")
                return f

            def mk_mm(nm, ib_, c_):
                def f():
                    if ib_ == 0:
                        hold["pacc"] = ps_a.tile(
                            [128, 512], F32, tag="ps_a", name="pacc")
                    nc.tensor.matmul(
                        hold["pacc"][:],
                        w_tiles[nm][:, ib_ * 128:(ib_ + 1) * 128],
                        xt[ib_][:, c_ * 512:(c_ + 1) * 512],
                        start=(ib_ == 0), stop=(ib_ == 7),
                    )
                return f

            def mk_cp(nm, c_):
                def f():
                    dst = proj_out[nm][:, c_ * 512:(c_ + 1) * 512]
                    if with_bias:
                        nc.vector.tensor_scalar_add(
                            dst, hold["pacc"][:], bias[nm][:])
                    else:
                        nc.vector.tensor_copy(dst, hold["pacc"][:])
                return f

            for name in ("q", "k", "v"):
                fq.push("u", mk_proj_alloc(name))
                for c in range(NC):
                    for ib in range(8):
                        fq.push("b" if ib == 0 else "u", mk_mm(name, ib, c))
                    fq.push("e", mk_cp(name, c))

        def ve_jobs(fq, b, proj_src, ve_out):
            """V natural [k, hd] with ones cols: ve2 layout per kb block of
            130 cols: [v_h0(64) | one | v_h1(64) | one]. Lazy tile allocs."""
            hold = {}

            def ones_cp():
                ve2 = vepool.tile([128, NB * 130], BF16, tag="ve2")
                ve_out.append(ve2)
                view = ve2[:].rearrange("p (kb h c) -> p kb h c", h=2, c=65)
                nc.vector.tensor_copy(
                    view[:, :, :, 64:65],
                    ones32[:].rearrange("p (kb h o) -> p kb h o", h=2, o=1),
                )
            fq.push("u", ones_cp)
            for g in range(4):  # groups of 4 s-blocks

                def mk_tr(g_, i_):
                    def f():
                        if i_ == 0:
                            hold["pst"] = ps_a.tile(
                                [128, 512], BF16, tag="ps_a", name="vtp")
                        sb = g_ * 4 + i_
                        nc.tensor.transpose(
                            hold["pst"][:, i_ * 128:(i_ + 1) * 128],
                            proj_src["v"][:, sb * 128:(sb + 1) * 128],
                            ident_b[:],
                        )
                    return f

                def mk_cp(g_):
                    def f():
                        view = ve_out[0][:].rearrange(
                            "p (kb h c) -> p kb h c", h=2, c=65)
                        nc.vector.tensor_copy(
                            view[:, g_ * 4:(g_ + 1) * 4, :, 0:64],
                            hold["pst"][:].rearrange(
                                "p (s hh cc) -> p s hh cc", s=4, cc=64),
                        )
                    return f

                for i in range(4):
                    fq.push("b" if i == 0 else "u", mk_tr(g, i))
                fq.push("e", mk_cp(g))

        def oproj_jobs(fq, b, aot_src):
            hold = {}
            for ob in range(8):
                for c in range(NC):

                    def mk_mm(ob_, c_):
                        def f():
                            if c_ == 0:
                                hold["stg"] = stgpool.tile(
                                    [128, S], BF16, tag="stg", name="stg")
                            hold["pst"] = ps_a.tile(
                                [128, 512], F32, tag="ps_a", name="pst")
                            nc.tensor.matmul(
                                hold["pst"][:],
                                wo_t[:, ob_ * 128:(ob_ + 1) * 128],
                                aot_src[0][:, c_ * 512:(c_ + 1) * 512],
                                start=True, stop=True,
                            )
                        return f

                    def mk_cp(c_):
                        def f():
                            nc.vector.tensor_copy(
                                hold["stg"][:, c_ * 512:(c_ + 1) * 512],
                                hold["pst"][:])
                        return f

                    fq.push("b", mk_mm(ob, c))
                    fq.push("e", mk_cp(c))

                def mk_dma(ob_):
                    def f():
                        nc.sync.dma_start(
                            td["out"][ob_ * 128:(ob_ + 1) * 128,
                                      b * S: b * S + S], hold["stg"][:])
                    return f
                fq.push("u", mk_dma(ob))

        def attention(b, qt, kt, vt, ve2, fq):
            """The latency-critical part, run inline with filler pops."""
            dn = nrmpool.tile([8, 512], BF16, tag="dn")
            aou = {}
            for c in range(NC):
                for h in range(HPC):
                    hp = slice(h * HD, (h + 1) * HD)
                    acc = ps_c.tile([128, 512], F32, tag="ps_c", name="acc")
                    n_kb = 4 * c + 4
                    pts = {}

                    def emit_scores(kb):
                        lo = max(0, 128 * kb - 512 * c)
                        st = ps_b.tile([128, 512], F32, tag="ps_b", name="st")
                        nc.tensor.matmul(
                            st[:, lo:512],
                            kt[hp, kb * 128:(kb + 1) * 128],
                            qt[hp, 512 * c + lo: 512 * (c + 1)],
                            start=True, stop=True,
                        )
                        pt = ptpool.tile([128, 512], BF16, tag="pt", name="pt")
                        nc.scalar.activation(
                            pt[:, lo:512], st[:, lo:512],
                            mybir.ActivationFunctionType.Exp,
                            scale=0.125,
                        )
                        if kb >= 4 * c:  # diagonal: mask (off critical path)
                            nc.gpsimd.tensor_mul(
                                pt[:, lo:lo + 128], pt[:, lo:lo + 128],
                                mask_b[:],
                            )
                        pts[kb] = pt

                    def emit_pv(kb):
                        lo = max(0, 128 * kb - 512 * c)
                        pt = pts.pop(kb)
                        last = kb == n_kb - 1
                        vsl = ve2[:, kb * 130 + 65 * h: kb * 130 + 65 * h + 65]
                        if kb >= 4 * c and lo + 128 < 512:
                            # unmasked columns first (independent of mask).
                            # start=True zero-marks the WHOLE psum bank, so
                            # exactly one start per accumulation chain.
                            nc.tensor.matmul(
                                acc[0:65, lo + 128:512], vsl,
                                pt[:, lo + 128:512],
                                start=(kb == 0), stop=False,
                            )
                            nc.tensor.matmul(
                                acc[0:65, lo:lo + 128], vsl,
                                pt[:, lo:lo + 128],
                                start=False, stop=last,
                            )
                        else:
                            nc.tensor.matmul(
                                acc[0:65, lo:512], vsl, pt[:, lo:512],
                                start=(kb == 0), stop=last,
                            )

                    for kb in range(min(LA, n_kb)):
                        emit_scores(kb)
                    for kb in range(n_kb):
                        if kb + LA < n_kb:
                            emit_scores(kb + LA)
                        # fillers BEFORE emit_pv: they execute in the PE's
                        # in-order stream while exp(kb) finishes on ACT
                        fq.pop(2)
                        emit_pv(kb)

                    # stash unnormalized out^T (DVE) + denom row
                    t_ = aoupool.tile([64, 512], BF16, tag="aou", name="aou")
                    nc.vector.tensor_copy(t_[:], acc[0:64, :])
                    aou[(h, c)] = t_
                    dstg = aoupool.tile([128, 512], BF16, tag="dstg",
                                        name="dstg", bufs=3)
                    nc.vector.tensor_copy(dstg[64:65, :], acc[64:65, :])
                    r = 4 * h + c
                    nc.sync.dma_start(dn[r:r + 1, :], dstg[64:65, :])
                    fq.pop(2)
            return dn, aou

        def normalize_jobs(fq, b, dn, aou, aot_out):
            """One reciprocal for the whole batch via transpose dance,
            pushed as filler jobs (hidden under the next batch's attention)."""
            hold = {}

            def mk_tr1(jblk):
                def f():
                    if jblk == 0:
                        hold["dnt_ps"] = ps_a.tile(
                            [128, 32], BF16, tag="ps_a", name="dnt")
                    nc.tensor.transpose(
                        hold["dnt_ps"][:, jblk * 8:(jblk + 1) * 8],
                        dn[0:8, jblk * 128:(jblk + 1) * 128],
                        ident_b[0:8, 0:8],
                    )
                return f

            def cp1():
                hold["dnt"] = nrmpool.tile([128, 32], F32, tag="dnt", name="dnt")
                nc.vector.tensor_copy(hold["dnt"][:], hold["dnt_ps"][:])

            def recip():
                hold["rdnt"] = nrmpool.tile([128, 32], BF16, tag="rdnt", name="rdnt")
                with nc.allow_low_precision(reason="bf16 softmax denoms"):
                    nc.vector.reciprocal(hold["rdnt"][:], hold["dnt"][:])

            def mk_tr2(jblk):
                def f():
                    if jblk == 0:
                        hold["rdn_ps"] = ps_a.tile(
                            [8, 512], BF16, tag="ps_a", name="rdnps")
                    nc.tensor.transpose(
                        hold["rdn_ps"][0:8, jblk * 128:(jblk + 1) * 128],
                        hold["rdnt"][:, jblk * 8:(jblk + 1) * 8],
                        ident_b[:],
                    )
                return f

            def cp2():
                hold["rdn"] = nrmpool.tile([8, 512], BF16, tag="rdn", name="rdn")
                nc.vector.tensor_copy(hold["rdn"][:], hold["rdn_ps"][:])
                aot_out.append(aotpool.tile([J, S], BF16, tag="aot", name="aot"))

            for jblk in range(4):
                fq.push("b" if jblk == 0 else "u", mk_tr1(jblk))
            fq.push("e", cp1)
            fq.push("u", recip)
            for jblk in range(4):
                fq.push("b" if jblk == 0 else "u", mk_tr2(jblk))
            fq.push("e", cp2)

            def mk_bc(h, c):
                def f():
                    hold["bcp"] = ps_a.tile(
                        [64, 512], F32, tag="ps_a", name="bcp")
                    r = 4 * h + c
                    nc.tensor.matmul(
                        hold["bcp"][:], sel[:, r * 64:(r + 1) * 64],
                        hold["rdn"][:], start=True, stop=True,
                    )
                return f

            def mk_mul(h, c):
                def f():
                    aot = aot_out[0]
                    if h == 0:
                        nc.vector.tensor_mul(
                            aot[0:64, c * 512:(c + 1) * 512],
                            aou[(h, c)][:], hold["bcp"][:],
                        )
                    else:
                        tmp = nrmpool.tile([64, 512], BF16, tag="tmp",
                                           name="tmp", bufs=3)
                        nc.vector.tensor_mul(
                            tmp[:], aou[(h, c)][:], hold["bcp"][:])
                        nc.sync.dma_start(
                            aot[64:128, c * 512:(c + 1) * 512], tmp[:],
                        )
                return f

            for c in range(NC):
                for h in range(HPC):
                    fq.push("b", mk_bc(h, c))
                    fq.push("e", mk_mul(h, c))

        # --- software-pipelined batch schedule ---------------------------
        # batch b attention interleaves: o_proj of b-1, then qkv+ve of b+1.
        xt = {0: load_xt(0)}
        proj = {}
        ve = {}
        aot = {}

        # prologue: batch 0 projections emitted directly
        fq0 = FillQueue()
        proj[0] = {}
        qkv_jobs(fq0, 0, xt[0], proj[0])
        ve[0] = []
        ve_jobs(fq0, 0, proj[0], ve[0])
        fq0.drain_all()

        def dump(name, src):
            if name in debug_outs:
                p, w = src.shape[0], src.shape[-1]
                nc.sync.dma_start(td[f"dbg_{name}"][0:p, 0:w], src)

        NBATCH = n_batches
        dns = {}
        aous = {}
        for b in range(NBATCH):
            fq = FillQueue()
            if b >= 1:
                aot[b - 1] = []
                normalize_jobs(fq, b - 1, dns[b - 1], aous[b - 1], aot[b - 1])
                oproj_jobs(fq, b - 1, aot[b - 1])
            if b + 1 < NBATCH:
                xt[b + 1] = load_xt(b + 1)
                proj[b + 1] = {}
                qkv_jobs(fq, b + 1, xt[b + 1], proj[b + 1])
                ve[b + 1] = []
                ve_jobs(fq, b + 1, proj[b + 1], ve[b + 1])
                xt.pop(b, None)

            if b == 0:
                for nm in ("q", "k", "v"):
                    dump(nm + "t", proj[0][nm][:])
                dump("ve2", ve[0][0][:])

            dns[b], aous[b] = attention(
                b, proj[b]["q"], proj[b]["k"], proj[b]["v"], ve[b][0], fq)
            fq.drain_all()
            if b == 0:
                dump("dn", dns[0][:])
            proj.pop(b, None)
            ve.pop(b, None)
            dns.pop(b - 1, None)
            aous.pop(b - 1, None)

        # epilogue: normalize + o_proj of the last batch
        fqz = FillQueue()
        bz = NBATCH - 1
        aot[bz] = []
        normalize_jobs(fqz, bz, dns[bz], aous[bz], aot[bz])
        oproj_jobs(fqz, bz, aot[bz])
        fqz.drain_all()


_NC_CACHE = {}


def _get_nc(with_bias=False):
    if with_bias not in _NC_CACHE:
        _NC_CACHE[with_bias] = build_kernel(with_bias)
    return _NC_CACHE[with_bias]


def make_in_maps(inputs):
    """Host-side prep: cast to bf16, pre-transpose x, pre-tile weights."""
    import ml_dtypes
    bf = ml_dtypes.bfloat16
    x = np.asarray(inputs["hidden_states"], np.float32).reshape(BS, D)
    xt = np.ascontiguousarray(x.T.astype(bf))  # [D, BS] bf16
    Wq = np.asarray(inputs["Wq"], np.float32)
    Wk = np.asarray(inputs["Wk"], np.float32)
    Wv = np.asarray(inputs["Wv"], np.float32)
    Wo = np.asarray(inputs["Wo"], np.float32)
    bq = np.asarray(inputs["bq"], np.float32)
    bk = np.asarray(inputs["bk"], np.float32)
    bv = np.asarray(inputs["bv"], np.float32)

    def wtile(W, js):
        # [D, 128] column slice -> lhsT tile layout [128, 1024]:
        # tile[p, ib*128 + j] = W[ib*128 + p, js.start + j]
        return np.ascontiguousarray(
            W[:, js].reshape(8, 128, 128).transpose(1, 0, 2).reshape(128, D)
            .astype(bf))

    cident = np.eye(128, dtype=np.float32).astype(bf)
    cmask = np.tril(np.ones((128, 128), np.float32)).T.astype(bf)
    cones = np.ones((128, 32), np.float32).astype(bf)
    csel = np.zeros((8, 512), np.float32)
    for r in range(8):
        csel[r, r * 64:(r + 1) * 64] = 1.0
    csel = csel.astype(bf)

    in_maps = []
    for c in range(N_CORES):
        js = slice(c * J, (c + 1) * J)
        in_maps.append({
            "xt": xt,
            "wq": wtile(Wq, js),
            "wk": wtile(Wk, js),
            "wv": wtile(Wv, js),
            "wo": np.ascontiguousarray(Wo[js, :].astype(bf)),
            "bq": np.ascontiguousarray(bq[js]),
            "bk": np.ascontiguousarray(bk[js]),
            "bv": np.ascontiguousarray(bv[js]),
            "cident": cident,
            "cmask": cmask,
            "cones": cones,
            "csel": csel,
        })
    return in_maps


def needs_bias(inputs):
    return any(
        np.any(np.asarray(inputs[k])) for k in ("bq", "bk", "bv"))


def gather_output(results, bo):
    out_t = np.zeros((D, BS), np.float32)
    for c in range(N_CORES):
        out_t += results[c]["out_t"].astype(np.float32)
    out = out_t.T + np.asarray(bo, np.float32)[None, :]
    return out.reshape(B, S, D)


def kernel(**inputs) -> np.ndarray:
    nc = _get_nc(needs_bias(inputs))
    in_maps = make_in_maps(inputs)
    res = run_bass_kernel_spmd(nc, in_maps, core_ids=list(range(N_CORES)))
    return gather_output(res.results, inputs["bo"])


if __name__ == "__main__":
    rng = np.random.default_rng(0)
    ins = {
        "hidden_states": rng.standard_normal((B, S, D), np.float32),
        "Wq": rng.standard_normal((D, D), np.float32) * 0.02,
        "bq": np.zeros(D, np.float32),
        "Wk": rng.standard_normal((D, D), np.float32) * 0.02,
        "bk": np.zeros(D, np.float32),
        "Wv": rng.standard_normal((D, D), np.float32) * 0.02,
        "bv": np.zeros(D, np.float32),
        "Wo": rng.standard_normal((D, D), np.float32) * 0.02,
        "bo": np.zeros(D, np.float32),
    }
    out = kernel(**ins)
    print("out", out.shape, out.dtype, float(np.abs(out).mean()))
